# revision 1
# baseline (speedup 1.0000x reference)
"""Distributed Trainium2 Bass kernel for nn_CrossAttention.

Reference computation (per batch b):
    q = x @ Wq.T + bq          (N, C)       C = 1024, H = 16 heads, D = 64
    k = enc @ Wk.T + bk        (T, C)
    v = enc @ Wv.T + bv        (T, C)
    att = softmax(q.k / sqrt(D))   per head
    y = (att @ v) @ Wp.T + bp  (N, C)

Sharding (8 cores): core c = (batch b = c//2, head-group g = c%2).
Each core owns 8 heads (512 channels) of Q/K/V for one batch, computes
attention for those heads, and a *partial* output projection using the
512 matching columns of Wp.  Host sums the two partials per batch and
adds bp.  No inter-core communication.

Device-side layout is "feature on partitions" throughout:
    Q^T, K^T : (512, N)  channel-major (projection computes W @ X^T),
               stored bf16 for the scores matmuls.
    V        : (T, 512)  token-major bf16, with a ones column appended per
               head so the attn@V matmul also emits the softmax denominator.
    S^T = K Q^T : (T-block=128 partitions, n free) fp32 in PSUM,
               exp(scale*s) on ScalarE -> P^T bf16.
    attn@V   : out = V'.T @ P^T -> (65, n) fp32 = [y^T ; denom],
               accumulated per 4-t-block slab into SBUF tiles.
    out-proj : O^T = Wp_cols^T.T @ Y^T  (partial, summed on host).

Projections and the output projection run as float32r (full-speed fp32
path on the TRN2 PE for moving dim >= 256).  Host pre-transposes inputs
so the device never transposes anything.
"""

import numpy as np
from contextlib import ExitStack

# ---------------------------------------------------------------- constants
B, N, T, C, H = 4, 2048, 2048, 1024, 16
G = 2                      # head groups (cores per batch)
N_CORES = 8
D = C // H                 # 64 head dim
HL = H // G                # 8 heads per core
CL = HL * D                # 512 local channels per core

_COMPILED = {}             # (aug_x, aug_e) -> compiled Bacc


def build(aug_x: bool, aug_e: bool, num_devices: int = N_CORES,
          n=N, t=T, c=C, hl=HL, d=D, repeat=1, proj_dtype="f32r"):
    """Build + compile the per-core SPMD program.  Parameterized so tests
    can build small versions for CoreSim (requires t == n), and repeat>1
    duplicates the body for wall-clock timing calibration."""
    import concourse.mybir as mybir
    import concourse.tile as tile
    from concourse import bacc

    f32 = mybir.dt.float32
    bf16 = mybir.dt.bfloat16
    f32r = mybir.dt.float32r
    pdt = {"f32r": f32r, "bf16": bf16}[proj_dtype]
    EXP = mybir.ActivationFunctionType.Exp

    cl = hl * d
    dp1 = d + 1
    KC = c // 128                      # contraction chunks (proj)
    NCH = n // 512                     # n chunks of 512
    TB = t // 128                      # t blocks of 128
    MQ = cl // 128                     # q/k channel blocks (== head pairs)
    MO = c // 128                      # output channel blocks
    PAIRS = hl // 2
    assert TB == NCH * 4 and MQ == PAIRS
    scale = 1.0 / float(np.sqrt(d))

    xrows = c + (1 if aug_x else 0)
    erows = c + (1 if aug_e else 0)
    kq_chunks = [(i * 128, 128) for i in range(KC)] + ([(c, 1)] if aug_x else [])
    ke_chunks = [(i * 128, 128) for i in range(KC)] + ([(c, 1)] if aug_e else [])

    nc = bacc.Bacc("TRN2", target_bir_lowering=False, debug=False,
                   enable_asserts=False, num_devices=num_devices)

    xt = nc.dram_tensor("xt", (xrows, n), pdt, kind="ExternalInput").ap()
    et = nc.dram_tensor("et", (erows, t), pdt, kind="ExternalInput").ap()
    wqt = nc.dram_tensor("wqt", (xrows, cl), pdt, kind="ExternalInput").ap()
    wkt = nc.dram_tensor("wkt", (erows, cl), pdt, kind="ExternalInput").ap()
    wvt = nc.dram_tensor("wvt", (erows, cl), pdt, kind="ExternalInput").ap()
    wpt = nc.dram_tensor("wpt", (cl, c), pdt, kind="ExternalInput").ap()
    ot = nc.dram_tensor("ot", (c, n), f32, kind="ExternalOutput").ap()

    def emit_body(tc):
        with ExitStack() as ctx:
            persist = ctx.enter_context(tc.tile_pool(name="persist", bufs=1))
            psum = ctx.enter_context(tc.tile_pool(name="psum", bufs=2,
                                                  space="PSUM"))
            spool = ctx.enter_context(tc.tile_pool(name="satt", bufs=2))

            qt = [persist.tile([128, n], bf16, name=f"qt{m}", tag=f"qt{m}")
                  for m in range(MQ)]
            kt = [persist.tile([128, t], bf16, name=f"kt{m}", tag=f"kt{m}")
                  for m in range(MQ)]
            vv = [persist.tile([128, hl * dp1], bf16, name=f"vv{i}",
                               tag=f"vv{i}") for i in range(TB)]
            wpt_sb = [persist.tile([128, c], pdt, name=f"wp{p}",
                                   tag=f"wp{p}") for p in range(PAIRS)]
            ones1 = persist.tile([dp1, d], f32r, name="ones1", tag="ones1")
            ones1f = persist.tile([dp1, d], f32, name="ones1f", tag="ones1f")
            nc.vector.memset(ones1f[d:dp1, :], 1.0)
            nc.vector.tensor_copy(ones1[d:dp1, :], ones1f[d:dp1, :])

            # --------------------------------------- phase KV (+ weights)
            kv_ctx = ExitStack()
            wk_pool = kv_ctx.enter_context(tc.tile_pool(name="wkp", bufs=1))
            es_pool = kv_ctx.enter_context(
                tc.tile_pool(name="esl", bufs=len(ke_chunks)))
            wk_sb, wv_sb = [], []

            def kv_iter(nt):
                """K^T and V for t-blocks 4nt..4nt+3.  The first iteration
                interleaves the weight-chunk loads with the es loads so the
                first matmul isn't stuck behind bulk DMA."""
                es = []
                for ki, (off, sz) in enumerate(ke_chunks):
                    if nt == 0:
                        wkc = wk_pool.tile([sz, cl], pdt, name=f"wkc{ki}",
                                           tag=f"wkc{ki}")
                        nc.sync.dma_start(wkc, wkt[off:off + sz, :])
                        wk_sb.append(wkc)
                        wvc = wk_pool.tile([sz, cl], pdt, name=f"wvc{ki}",
                                           tag=f"wvc{ki}")
                        nc.sync.dma_start(wvc, wvt[off:off + sz, :])
                        wv_sb.append(wvc)
                    e = es_pool.tile([128, 512], pdt, name="es", tag="es")
                    nc.sync.dma_start(
                        e[:sz, :], et[off:off + sz, nt * 512:(nt + 1) * 512])
                    es.append(e)
                nk = len(ke_chunks)
                for mh in (range(0, MQ, 2) if MQ > 1 else [0]):
                    ms = [m for m in (mh, mh + 1) if m < MQ]
                    ps = [psum.tile([128, 512], f32, name=f"pk{m}", tag="pa")
                          for m in ms]
                    for ki, (off, sz) in enumerate(ke_chunks):
                        for j, m in enumerate(ms):
                            nc.tensor.matmul(
                                ps[j], wk_sb[ki][:, m * 128:(m + 1) * 128],
                                es[ki][:sz, :],
                                start=(ki == 0), stop=(ki == nk - 1))
                    for j, m in enumerate(ms):
                        nc.vector.tensor_copy(
                            kt[m][:, nt * 512:(nt + 1) * 512], ps[j])
                for th in (0, 2):
                    ps = [psum.tile([128, cl], f32, name=f"pv{tb}", tag="pa")
                          for tb in (th, th + 1)]
                    for ki, (off, sz) in enumerate(ke_chunks):
                        for j, tb in enumerate((th, th + 1)):
                            nc.tensor.matmul(
                                ps[j], es[ki][:sz, tb * 128:(tb + 1) * 128],
                                wv_sb[ki],
                                start=(ki == 0), stop=(ki == nk - 1))
                    for j, tb in enumerate((th, th + 1)):
                        ti = nt * 4 + tb
                        src = ps[j].rearrange("p (h e) -> p h e", h=hl)
                        dst = vv[ti].rearrange("p (h e) -> p h e", h=hl)
                        nc.vector.tensor_copy(dst[:, :, 0:d], src)
                        nc.vector.memset(dst[:, :, d:dp1], 1.0)

            # --------------------------------------- phase Q (+ weights)
            q_ctx = ExitStack()
            wq_pool = q_ctx.enter_context(tc.tile_pool(name="wqp", bufs=1))
            xs_pool = q_ctx.enter_context(
                tc.tile_pool(name="xsl", bufs=len(kq_chunks)))
            wq_sb = []

            def q_iter(nq):
                xs = []
                for ki, (off, sz) in enumerate(kq_chunks):
                    if nq == 0:
                        wqc = wq_pool.tile([sz, cl], pdt, name=f"wqc{ki}",
                                           tag=f"wqc{ki}")
                        nc.sync.dma_start(wqc, wqt[off:off + sz, :])
                        wq_sb.append(wqc)
                    x = xs_pool.tile([128, 512], pdt, name="xs", tag="xs")
                    nc.sync.dma_start(
                        x[:sz, :], xt[off:off + sz, nq * 512:(nq + 1) * 512])
                    xs.append(x)
                nk = len(kq_chunks)
                for mh in (range(0, MQ, 2) if MQ > 1 else [0]):
                    ms = [m for m in (mh, mh + 1) if m < MQ]
                    ps = [psum.tile([128, 512], f32, name=f"pq{m}", tag="pa")
                          for m in ms]
                    for ki, (off, sz) in enumerate(kq_chunks):
                        for j, m in enumerate(ms):
                            nc.tensor.matmul(
                                ps[j], wq_sb[ki][:, m * 128:(m + 1) * 128],
                                xs[ki][:sz, :],
                                start=(ki == 0), stop=(ki == nk - 1))
                    for j, m in enumerate(ms):
                        nc.vector.tensor_copy(
                            qt[m][:, nq * 512:(nq + 1) * 512], ps[j])

            # --------------------------------------- attention
            def att_pair(nq, p, av, trange, first_slab):
                """Scores + exp + attn@V for head pair p of n-chunk nq over
                the t-blocks in trange (a slab).  The slab's attn@V partial
                lives in a short-lived PSUM tile and folds into the SBUF
                accumulators av, so PSUM av slots never block on the
                normalize chain."""
                h0, h1 = 2 * p, 2 * p + 1
                trange = list(trange)
                avp = (psum.tile([dp1, 512], f32, name="avp0", tag="av0",
                                 bufs=1),
                       psum.tile([dp1, 512], f32, name="avp1", tag="av1",
                                 bufs=1))
                for ti in trange:
                    sc = psum.tile([128, 1024], f32, name="sc", tag="sc2")
                    nc.tensor.matmul(
                        sc[:, 0:512],
                        kt[p][0:64, ti * 128:(ti + 1) * 128],
                        qt[p][0:64, nq * 512:(nq + 1) * 512],
                        start=True, stop=True)
                    nc.tensor.matmul(
                        sc[:, 512:1024],
                        kt[p][64:128, ti * 128:(ti + 1) * 128],
                        qt[p][64:128, nq * 512:(nq + 1) * 512],
                        start=True, stop=True)
                    pt = spool.tile([128, 1024], bf16, name="pt", tag="pt",
                                    bufs=3)
                    nc.scalar.activation(pt, sc, EXP, scale=scale)
                    nc.tensor.matmul(
                        avp[0], vv[ti][:, h0 * dp1:(h0 + 1) * dp1],
                        pt[:, 0:512],
                        start=(ti == trange[0]), stop=(ti == trange[-1]))
                    nc.tensor.matmul(
                        avp[1], vv[ti][:, h1 * dp1:(h1 + 1) * dp1],
                        pt[:, 512:1024],
                        start=(ti == trange[0]), stop=(ti == trange[-1]))
                for j in range(2):
                    if first_slab:
                        nc.vector.tensor_copy(av[j], avp[j])
                    else:
                        nc.vector.tensor_add(av[j], av[j], avp[j])

            def att_recip(av):
                """Early half of normalize: DVE reciprocals of the softmax
                denominators.  Emitted right after the pair's attention so
                the result is long ready when the PE broadcast runs."""
                rcs = []
                for j in range(2):
                    rc = spool.tile([dp1, 512], f32r, name="rc", tag="rc",
                                    bufs=6)
                    with nc.allow_low_precision(reason="f32r == f32 bits"):
                        nc.vector.reciprocal(rc[d:dp1, :], av[j][d:dp1, :])
                    rcs.append(rc)
                return rcs

            def att_finish(rcs, av, ytp):
                """Late half: PE broadcast of 1/denom, then y^T = av * R."""
                for j in range(2):
                    Rp = psum.tile([64, 512], f32, name="Rp", tag="sc2")
                    nc.tensor.matmul(Rp, ones1[d:dp1, :], rcs[j][d:dp1, :],
                                     start=True, stop=True)
                    if j == 0:
                        nc.vector.tensor_mul(ytp[0:64, :], av[j][0:d, :], Rp)
                    else:
                        ytm = spool.tile([64, 512], pdt, name="ytm",
                                         tag="ytm")
                        nc.vector.tensor_mul(ytm, av[j][0:d, :], Rp)
                        nc.sync.dma_start(ytp[64:128, :], ytm)

            def out_proj(nq, yts):
                for m in range(MO):
                    po = psum.tile([128, 512], f32, name="po", tag="pa")
                    for p in range(PAIRS):
                        nc.tensor.matmul(
                            po, wpt_sb[p][:, m * 128:(m + 1) * 128], yts[p],
                            start=(p == 0), stop=(p == PAIRS - 1))
                    ob = spool.tile([128, 512], f32, name="ob", tag="ob",
                                    bufs=2)
                    nc.vector.tensor_copy(ob, po)
                    nc.sync.dma_start(ot[m * 128:(m + 1) * 128,
                                         nq * 512:(nq + 1) * 512], ob)

            # SBUF attn@V accumulators, shared across n-chunks.
            av_sb = [(persist.tile([dp1, 512], f32, name=f"avs{p}0",
                                   tag=f"avs{p}0"),
                      persist.tile([dp1, 512], f32, name=f"avs{p}1",
                                   tag=f"avs{p}1"))
                     for p in range(PAIRS)]

            # Program order hand-interleaves phases KV/Q with ALL of
            # n-chunk 0's attention so ScalarE (exp) starts as soon as the
            # first K/V slab lands.  Normalize is software-pipelined ~2
            # attention units behind the pair that produced it (reciprocal
            # emitted immediately, PE broadcast + muls later) so the PE
            # never stalls on the DVE chain.
            yts_by = {nqi: [] for nqi in range(NCH)}
            pending = []

            def flush_one():
                rcs, av, nq2, _p2 = pending.pop(0)
                ytp = spool.tile([128, 512], pdt, name="ytp", tag="ytp",
                                 bufs=PAIRS + 2)
                att_finish(rcs, av, ytp)
                yts_by[nq2].append(ytp)
                if len(yts_by[nq2]) == PAIRS:
                    out_proj(nq2, yts_by[nq2])

            for nt in range(NCH):
                kv_iter(nt)
                q_iter(nt)
                if nt == 0:
                    for p in range(PAIRS):
                        nc.sync.dma_start(wpt_sb[p],
                                          wpt[p * 128:(p + 1) * 128, :])
                for p in range(PAIRS):
                    att_pair(0, p, av_sb[p], range(nt * 4, nt * 4 + 4),
                             first_slab=(nt == 0))
                    if nt == NCH - 1:
                        pending.append((att_recip(av_sb[p]), av_sb[p], 0, p))
            q_ctx.close()
            kv_ctx.close()

            # Flush BEFORE each attention unit: stageB(nq, p) must be
            # emitted before att_pair(nq+1, p) overwrites av_sb[p], and
            # pipeline depth stays <= 2 so rc tiles bound.
            for nq in range(1, NCH):
                for p in range(PAIRS):
                    while (len(pending) > 2
                           or any(e[3] == p for e in pending)):
                        flush_one()
                    for si in range(TB // 4):
                        att_pair(nq, p, av_sb[p], range(si * 4, si * 4 + 4),
                                 first_slab=(si == 0))
                    pending.append((att_recip(av_sb[p]), av_sb[p], nq, p))
            while pending:
                flush_one()

    with tile.TileContext(nc) as tc:
        for _rep in range(repeat):
            emit_body(tc)

    nc.compile()
    return nc


def _get_compiled(aug_x: bool, aug_e: bool):
    key = (aug_x, aug_e)
    if key not in _COMPILED:
        _COMPILED[key] = build(aug_x, aug_e)
    return _COMPILED[key]


def shard_inputs(x, enc, Wq, bq, Wk, bk, Wv, bv, Wp, aug_x, aug_e,
                 g_groups=G, cl=CL, proj_dtype="f32r"):
    if proj_dtype == "bf16":
        import ml_dtypes
        npdt = ml_dtypes.bfloat16
    else:
        npdt = np.float32
    in_maps = []
    n_cores = x.shape[0] * g_groups
    onesN = np.ones((1, x.shape[1]), np.float32)
    onesT = np.ones((1, enc.shape[1]), np.float32)
    for core in range(n_cores):
        b, g = divmod(core, g_groups)
        sl = slice(g * cl, (g + 1) * cl)
        xtc = x[b].T
        etc = enc[b].T
        wqtc = Wq[sl, :].T
        wktc = Wk[sl, :].T
        wvtc = Wv[sl, :].T
        if aug_x:
            xtc = np.concatenate([xtc, onesN], axis=0)
            wqtc = np.concatenate([wqtc, bq[sl][None, :]], axis=0)
        if aug_e:
            etc = np.concatenate([etc, onesT], axis=0)
            wktc = np.concatenate([wktc, bk[sl][None, :]], axis=0)
            wvtc = np.concatenate([wvtc, bv[sl][None, :]], axis=0)
        in_maps.append({
            "xt": np.ascontiguousarray(xtc, npdt),
            "et": np.ascontiguousarray(etc, npdt),
            "wqt": np.ascontiguousarray(wqtc, npdt),
            "wkt": np.ascontiguousarray(wktc, npdt),
            "wvt": np.ascontiguousarray(wvtc, npdt),
            "wpt": np.ascontiguousarray(Wp[:, sl].T, npdt),
        })
    return in_maps


def run_spmd(in_maps, nc=None, aug_x=False, aug_e=False, **kw):
    from concourse import bass_utils
    if nc is None:
        nc = _get_compiled(aug_x, aug_e)
    return bass_utils.run_bass_kernel_spmd(
        nc, in_maps, core_ids=list(range(len(in_maps))), **kw)


def kernel(**inputs):
    x = np.asarray(inputs["x"], np.float32)
    enc = np.asarray(inputs["encoder_output"], np.float32)
    Wq = np.asarray(inputs["Wq"], np.float32)
    bq = np.asarray(inputs["bq"], np.float32)
    Wk = np.asarray(inputs["Wk"], np.float32)
    bk = np.asarray(inputs["bk"], np.float32)
    Wv = np.asarray(inputs["Wv"], np.float32)
    bv = np.asarray(inputs["bv"], np.float32)
    Wp = np.asarray(inputs["Wp"], np.float32)
    bp = np.asarray(inputs["bp"], np.float32)

    aug_x = bool(np.any(bq))
    aug_e = bool(np.any(bk)) or bool(np.any(bv))
    nc = _get_compiled(aug_x, aug_e)
    in_maps = shard_inputs(x, enc, Wq, bq, Wk, bk, Wv, bv, Wp, aug_x, aug_e)
    res = run_spmd(in_maps, nc=nc)
    y = np.empty((B, N, C), np.float32)
    for b in range(B):
        y[b] = (res.results[2 * b]["ot"] +
                res.results[2 * b + 1]["ot"]).T + bp[None, :]
    return y



# revision 9
# speedup vs baseline: 1.1996x; 1.1996x over previous
"""Distributed Trainium2 Bass kernel for nn_CrossAttention.

Reference computation (per batch b):
    q = x @ Wq.T + bq          (N, C)       C = 1024, H = 16 heads, D = 64
    k = enc @ Wk.T + bk        (T, C)
    v = enc @ Wv.T + bv        (T, C)
    att = softmax(q.k / sqrt(D))   per head
    y = (att @ v) @ Wp.T + bp  (N, C)

Sharding (8 cores): core c = (batch b = c//2, head-group g = c%2).
Each core owns 8 heads (512 channels) of Q/K/V for one batch, computes
attention for those heads, and a *partial* output projection using the
512 matching columns of Wp.  Host sums the two partials per batch and
adds bp.  No inter-core communication.

Device-side layout is "feature on partitions" throughout:
    Q^T, K^T : (512, N)  channel-major (projection computes W @ X^T),
               stored bf16 for the scores matmuls.
    V        : (T, 512)  token-major bf16, with a ones column appended per
               head so the attn@V matmul also emits the softmax denominator.
    S^T = K Q^T : (T-block=128 partitions, n free) fp32 in PSUM,
               exp(scale*s) on ScalarE -> P^T bf16.
    attn@V   : out = V'.T @ P^T -> (65, n) fp32 = [y^T ; denom],
               accumulated per 4-t-block slab into SBUF tiles.
    out-proj : O^T = Wp_cols^T.T @ Y^T  (partial, summed on host).

Projections stream the activations as float32r (full-speed fp32 PE path
for moving dim >= 256) against bf16 stationary weights.  Softmax
normalization uses reciprocal_approx_fast (DVE) + a PE ones-broadcast;
the attn@V accumulators are double-buffered by nq parity so the
normalize chain is never on the PE critical path.
"""

import numpy as np
from contextlib import ExitStack

# ---------------------------------------------------------------- constants
B, N, T, C, H = 4, 2048, 2048, 1024, 16
G = 2                      # head groups (cores per batch)
N_CORES = 8
D = C // H                 # 64 head dim
HL = H // G                # 8 heads per core
CL = HL * D                # 512 local channels per core

_COMPILED = {}             # (aug_x, aug_e) -> compiled Bacc


def build(aug_x: bool, aug_e: bool, num_devices: int = N_CORES,
          n=N, t=T, c=C, hl=HL, d=D, repeat=1, proj_dtype="bf16",
          weight_dtype="bf16"):
    """Build + compile the per-core SPMD program.  Parameterized so tests
    can build small versions for CoreSim (requires t == n), and repeat>1
    duplicates the body for wall-clock timing calibration."""
    import concourse.mybir as mybir
    import concourse.tile as tile
    from concourse import bacc

    f32 = mybir.dt.float32
    bf16 = mybir.dt.bfloat16
    f32r = mybir.dt.float32r
    pdt = {"f32r": f32r, "bf16": bf16}[proj_dtype]
    wdt = {"f32r": f32r, "bf16": bf16}[weight_dtype]
    EXP = mybir.ActivationFunctionType.Exp

    cl = hl * d
    dp1 = d + 1
    KC = c // 128                      # contraction chunks (proj)
    NCH = n // 512                     # n chunks of 512
    TB = t // 128                      # t blocks of 128
    MQ = cl // 128                     # q/k channel blocks (== head pairs)
    MO = c // 128                      # output channel blocks
    PAIRS = hl // 2
    assert TB == NCH * 4 and MQ == PAIRS
    scale = 1.0 / float(np.sqrt(d))

    xrows = c + (1 if aug_x else 0)
    erows = c + (1 if aug_e else 0)
    kq_chunks = [(i * 128, 128) for i in range(KC)] + ([(c, 1)] if aug_x else [])
    ke_chunks = [(i * 128, 128) for i in range(KC)] + ([(c, 1)] if aug_e else [])

    nc = bacc.Bacc("TRN2", target_bir_lowering=False, debug=False,
                   enable_asserts=False, num_devices=num_devices)

    xt = nc.dram_tensor("xt", (xrows, n), pdt, kind="ExternalInput").ap()
    et = nc.dram_tensor("et", (erows, t), pdt, kind="ExternalInput").ap()
    wqt = nc.dram_tensor("wqt", (xrows, cl), wdt, kind="ExternalInput").ap()
    wkt = nc.dram_tensor("wkt", (erows, cl), wdt, kind="ExternalInput").ap()
    wvt = nc.dram_tensor("wvt", (erows, cl), wdt, kind="ExternalInput").ap()
    wpt = nc.dram_tensor("wpt", (cl, c), wdt, kind="ExternalInput").ap()
    ot = nc.dram_tensor("ot", (c, n), f32, kind="ExternalOutput").ap()

    def emit_body(tc):
        with ExitStack() as ctx:
            persist = ctx.enter_context(tc.tile_pool(name="persist", bufs=1))
            psum = ctx.enter_context(tc.tile_pool(name="psum", bufs=2,
                                                  space="PSUM"))
            spool = ctx.enter_context(tc.tile_pool(name="satt", bufs=2))

            qt = [persist.tile([128, n], bf16, name=f"qt{m}", tag=f"qt{m}")
                  for m in range(MQ)]
            kt = [persist.tile([128, t], bf16, name=f"kt{m}", tag=f"kt{m}")
                  for m in range(MQ)]
            vv = [persist.tile([128, hl * dp1], bf16, name=f"vv{i}",
                               tag=f"vv{i}") for i in range(TB)]
            wpt_sb = [persist.tile([128, c], wdt, name=f"wp{p}",
                                   tag=f"wp{p}") for p in range(PAIRS)]
            ones1 = persist.tile([1, d], f32r, name="ones1", tag="ones1")
            ones1f = persist.tile([1, d], f32, name="ones1f", tag="ones1f")
            nc.vector.memset(ones1f, 1.0)
            with nc.allow_low_precision(reason="f32r ones"):
                nc.vector.tensor_copy(ones1, ones1f)

            # --------------------------------------- phase KV (+ weights)
            kv_ctx = ExitStack()
            wk_pool = kv_ctx.enter_context(tc.tile_pool(name="wkp", bufs=1))
            es_pool = kv_ctx.enter_context(
                tc.tile_pool(name="esl", bufs=len(ke_chunks)))
            wk_sb, wv_sb = [], []

            def kv_iter(nt):
                """K^T and V for t-blocks 4nt..4nt+3.  The first iteration
                interleaves the weight-chunk loads with the es loads so the
                first matmul isn't stuck behind bulk DMA."""
                es = []
                for ki, (off, sz) in enumerate(ke_chunks):
                    if nt == 0:
                        wkc = wk_pool.tile([sz, cl], wdt, name=f"wkc{ki}",
                                           tag=f"wkc{ki}")
                        nc.sync.dma_start(wkc, wkt[off:off + sz, :])
                        wk_sb.append(wkc)
                        wvc = wk_pool.tile([sz, cl], wdt, name=f"wvc{ki}",
                                           tag=f"wvc{ki}")
                        nc.sync.dma_start(wvc, wvt[off:off + sz, :])
                        wv_sb.append(wvc)
                    e = es_pool.tile([128, 512], pdt, name="es", tag="es")
                    nc.sync.dma_start(
                        e[:sz, :], et[off:off + sz, nt * 512:(nt + 1) * 512])
                    es.append(e)
                nk = len(ke_chunks)
                for mh in (range(0, MQ, 2) if MQ > 1 else [0]):
                    ms = [m for m in (mh, mh + 1) if m < MQ]
                    ps = [psum.tile([128, 512], f32, name=f"pk{m}", tag="pa")
                          for m in ms]
                    for ki, (off, sz) in enumerate(ke_chunks):
                        for j, m in enumerate(ms):
                            nc.tensor.matmul(
                                ps[j], wk_sb[ki][:, m * 128:(m + 1) * 128],
                                es[ki][:sz, :],
                                start=(ki == 0), stop=(ki == nk - 1))
                    for j, m in enumerate(ms):
                        nc.vector.tensor_copy(
                            kt[m][:, nt * 512:(nt + 1) * 512], ps[j])
                for th in (0, 2):
                    ps = [psum.tile([128, cl], f32, name=f"pv{tb}", tag="pa")
                          for tb in (th, th + 1)]
                    for ki, (off, sz) in enumerate(ke_chunks):
                        for j, tb in enumerate((th, th + 1)):
                            nc.tensor.matmul(
                                ps[j], es[ki][:sz, tb * 128:(tb + 1) * 128],
                                wv_sb[ki],
                                start=(ki == 0), stop=(ki == nk - 1))
                    for j, tb in enumerate((th, th + 1)):
                        ti = nt * 4 + tb
                        src = ps[j].rearrange("p (h e) -> p h e", h=hl)
                        dst = vv[ti].rearrange("p (h e) -> p h e", h=hl)
                        nc.vector.tensor_copy(dst[:, :, 0:d], src)
                        nc.vector.memset(dst[:, :, d:dp1], 1.0)

            # --------------------------------------- phase Q (+ weights)
            q_ctx = ExitStack()
            wq_pool = q_ctx.enter_context(tc.tile_pool(name="wqp", bufs=1))
            xs_pool = q_ctx.enter_context(
                tc.tile_pool(name="xsl", bufs=len(kq_chunks)))
            wq_sb = []

            def q_iter(nq):
                xs = []
                for ki, (off, sz) in enumerate(kq_chunks):
                    if nq == 0:
                        wqc = wq_pool.tile([sz, cl], wdt, name=f"wqc{ki}",
                                           tag=f"wqc{ki}")
                        nc.sync.dma_start(wqc, wqt[off:off + sz, :])
                        wq_sb.append(wqc)
                    x = xs_pool.tile([128, 512], pdt, name="xs", tag="xs")
                    nc.sync.dma_start(
                        x[:sz, :], xt[off:off + sz, nq * 512:(nq + 1) * 512])
                    xs.append(x)
                nk = len(kq_chunks)
                for mh in (range(0, MQ, 2) if MQ > 1 else [0]):
                    ms = [m for m in (mh, mh + 1) if m < MQ]
                    ps = [psum.tile([128, 512], f32, name=f"pq{m}", tag="pa")
                          for m in ms]
                    for ki, (off, sz) in enumerate(kq_chunks):
                        for j, m in enumerate(ms):
                            nc.tensor.matmul(
                                ps[j], wq_sb[ki][:, m * 128:(m + 1) * 128],
                                xs[ki][:sz, :],
                                start=(ki == 0), stop=(ki == nk - 1))
                    for j, m in enumerate(ms):
                        nc.vector.tensor_copy(
                            qt[m][:, nq * 512:(nq + 1) * 512], ps[j])

            # --------------------------------------- attention
            def att_pair(nq, p, av, trange, first_slab):
                """Scores + exp + attn@V for head pair p of n-chunk nq over
                the t-blocks in trange (a slab).  The slab's attn@V partial
                lives in a short-lived PSUM tile and folds into the SBUF
                accumulators av."""
                h0, h1 = 2 * p, 2 * p + 1
                trange = list(trange)
                avp = (psum.tile([dp1, 512], f32, name="avp0", tag="av0",
                                 bufs=1),
                       psum.tile([dp1, 512], f32, name="avp1", tag="av1",
                                 bufs=1))
                for ti in trange:
                    sc = psum.tile([128, 1024], f32, name="sc", tag="sc2")
                    nc.tensor.matmul(
                        sc[:, 0:512],
                        kt[p][0:64, ti * 128:(ti + 1) * 128],
                        qt[p][0:64, nq * 512:(nq + 1) * 512],
                        start=True, stop=True)
                    nc.tensor.matmul(
                        sc[:, 512:1024],
                        kt[p][64:128, ti * 128:(ti + 1) * 128],
                        qt[p][64:128, nq * 512:(nq + 1) * 512],
                        start=True, stop=True)
                    pt = spool.tile([128, 1024], bf16, name="pt", tag="pt",
                                    bufs=3)
                    nc.scalar.activation(pt, sc, EXP, scale=scale)
                    nc.tensor.matmul(
                        avp[0], vv[ti][:, h0 * dp1:(h0 + 1) * dp1],
                        pt[:, 0:512],
                        start=(ti == trange[0]), stop=(ti == trange[-1]))
                    nc.tensor.matmul(
                        avp[1], vv[ti][:, h1 * dp1:(h1 + 1) * dp1],
                        pt[:, 512:1024],
                        start=(ti == trange[0]), stop=(ti == trange[-1]))
                for j in range(2):
                    if first_slab:
                        nc.vector.tensor_copy(av[j], avp[j])
                    else:
                        nc.vector.tensor_add(av[j], av[j], avp[j])

            def att_recip(av):
                """Early half of normalize: shift both heads' denominators
                to partition 0 (reciprocal_approx_fast misbehaves at
                nonzero base partitions on HW), fast-reciprocal, round to
                f32r for the PE broadcast.  Emitted right after the pair's
                attention so the result is long ready at flush time."""
                dn = spool.tile([1, 1024], f32, name="dn", tag="dn", bufs=2)
                for j in range(2):
                    nc.vector.tensor_copy(dn[:, j * 512:(j + 1) * 512],
                                          av[j][d:dp1, :])
                rcf = spool.tile([1, 1024], f32, name="rcf", tag="rcf",
                                 bufs=2)
                nc.vector.reciprocal_approx_fast(out=rcf, in_=dn)
                rc = spool.tile([1, 1024], f32r, name="rc", tag="rc",
                                bufs=6)
                with nc.allow_low_precision(reason="f32r == f32 bits"):
                    nc.vector.tensor_copy(rc, rcf)
                return rc

            def att_finish(rc, av, ytp):
                """Late half: PE broadcast of 1/denom, then y^T = av * R."""
                for j in range(2):
                    Rp = psum.tile([64, 512], f32, name="Rp", tag="sc2")
                    nc.tensor.matmul(Rp, ones1,
                                     rc[:, j * 512:(j + 1) * 512],
                                     start=True, stop=True)
                    if j == 0:
                        nc.vector.tensor_mul(ytp[0:64, :], av[j][0:d, :], Rp)
                    else:
                        ytm = spool.tile([64, 512], pdt, name="ytm",
                                         tag="ytm")
                        nc.vector.tensor_mul(ytm, av[j][0:d, :], Rp)
                        nc.sync.dma_start(ytp[64:128, :], ytm)

            def out_proj(nq, yts):
                for m in range(MO):
                    po = psum.tile([128, 512], f32, name="po", tag="pa")
                    for p in range(PAIRS):
                        nc.tensor.matmul(
                            po, wpt_sb[p][:, m * 128:(m + 1) * 128], yts[p],
                            start=(p == 0), stop=(p == PAIRS - 1))
                    ob = spool.tile([128, 512], f32, name="ob", tag="ob",
                                    bufs=2)
                    nc.vector.tensor_copy(ob, po)
                    nc.sync.dma_start(ot[m * 128:(m + 1) * 128,
                                         nq * 512:(nq + 1) * 512], ob)

            # SBUF attn@V accumulators, double-buffered by nq parity so a
            # unit's normalize chain has a full nq round before its tiles
            # are overwritten -- it is never on the PE critical path.
            av_sb = [[(persist.tile([dp1, 512], f32, name=f"av{s}_{p}0",
                                    tag=f"av{s}_{p}0"),
                       persist.tile([dp1, 512], f32, name=f"av{s}_{p}1",
                                    tag=f"av{s}_{p}1"))
                      for p in range(PAIRS)]
                     for s in range(min(2, NCH))]

            yts_by = {nqi: [] for nqi in range(NCH)}
            pending = []

            def flush_one():
                rc, av, nq2, _p2 = pending.pop(0)
                ytp = spool.tile([128, 512], pdt, name="ytp", tag="ytp",
                                 bufs=PAIRS + 2)
                att_finish(rc, av, ytp)
                yts_by[nq2].append(ytp)
                if len(yts_by[nq2]) == PAIRS:
                    out_proj(nq2, yts_by[nq2])

            # nq=0 attention is hand-interleaved with the KV/Q load+proj
            # phases so ScalarE (exp) starts as soon as the first K/V slab
            # lands.
            for nt in range(NCH):
                kv_iter(nt)
                q_iter(nt)
                if nt == 0:
                    for p in range(PAIRS):
                        nc.sync.dma_start(wpt_sb[p],
                                          wpt[p * 128:(p + 1) * 128, :])
                for p in range(PAIRS):
                    att_pair(0, p, av_sb[0][p], range(nt * 4, nt * 4 + 4),
                             first_slab=(nt == 0))
                    if nt == NCH - 1:
                        pending.append((att_recip(av_sb[0][p]),
                                        av_sb[0][p], 0, p))
            q_ctx.close()
            kv_ctx.close()

            # Steady state: before unit (nq, p), finish pair (nq-1, p).
            # The finish's reciprocal was computed a full nq round earlier,
            # so the DVE work here is just the two muls.
            for nq in range(1, NCH):
                for p in range(PAIRS):
                    flush_one()
                    av = av_sb[nq % 2][p]
                    for si in range(TB // 4):
                        att_pair(nq, p, av, range(si * 4, si * 4 + 4),
                                 first_slab=(si == 0))
                    pending.append((att_recip(av), av, nq, p))
            while pending:
                flush_one()

    with tile.TileContext(nc) as tc:
        for _rep in range(repeat):
            emit_body(tc)

    nc.compile()
    return nc


def _get_compiled(aug_x: bool, aug_e: bool):
    key = (aug_x, aug_e)
    if key not in _COMPILED:
        _COMPILED[key] = build(aug_x, aug_e)
    return _COMPILED[key]


def shard_inputs(x, enc, Wq, bq, Wk, bk, Wv, bv, Wp, aug_x, aug_e,
                 g_groups=G, cl=CL, proj_dtype="bf16", weight_dtype="bf16"):
    import ml_dtypes
    npdt = (ml_dtypes.bfloat16 if proj_dtype == "bf16" else np.float32)
    nwdt = (ml_dtypes.bfloat16 if weight_dtype == "bf16" else np.float32)
    in_maps = []
    n_cores = x.shape[0] * g_groups
    onesN = np.ones((1, x.shape[1]), np.float32)
    onesT = np.ones((1, enc.shape[1]), np.float32)
    for core in range(n_cores):
        b, g = divmod(core, g_groups)
        sl = slice(g * cl, (g + 1) * cl)
        xtc = x[b].T
        etc = enc[b].T
        wqtc = Wq[sl, :].T
        wktc = Wk[sl, :].T
        wvtc = Wv[sl, :].T
        if aug_x:
            xtc = np.concatenate([xtc, onesN], axis=0)
            wqtc = np.concatenate([wqtc, bq[sl][None, :]], axis=0)
        if aug_e:
            etc = np.concatenate([etc, onesT], axis=0)
            wktc = np.concatenate([wktc, bk[sl][None, :]], axis=0)
            wvtc = np.concatenate([wvtc, bv[sl][None, :]], axis=0)
        in_maps.append({
            "xt": np.ascontiguousarray(xtc, npdt),
            "et": np.ascontiguousarray(etc, npdt),
            "wqt": np.ascontiguousarray(wqtc, nwdt),
            "wkt": np.ascontiguousarray(wktc, nwdt),
            "wvt": np.ascontiguousarray(wvtc, nwdt),
            "wpt": np.ascontiguousarray(Wp[:, sl].T, nwdt),
        })
    return in_maps


def run_spmd(in_maps, nc=None, aug_x=False, aug_e=False, **kw):
    from concourse import bass_utils
    if nc is None:
        nc = _get_compiled(aug_x, aug_e)
    return bass_utils.run_bass_kernel_spmd(
        nc, in_maps, core_ids=list(range(len(in_maps))), **kw)


def kernel(**inputs):
    x = np.asarray(inputs["x"], np.float32)
    enc = np.asarray(inputs["encoder_output"], np.float32)
    Wq = np.asarray(inputs["Wq"], np.float32)
    bq = np.asarray(inputs["bq"], np.float32)
    Wk = np.asarray(inputs["Wk"], np.float32)
    bk = np.asarray(inputs["bk"], np.float32)
    Wv = np.asarray(inputs["Wv"], np.float32)
    bv = np.asarray(inputs["bv"], np.float32)
    Wp = np.asarray(inputs["Wp"], np.float32)
    bp = np.asarray(inputs["bp"], np.float32)

    aug_x = bool(np.any(bq))
    aug_e = bool(np.any(bk)) or bool(np.any(bv))
    nc = _get_compiled(aug_x, aug_e)
    in_maps = shard_inputs(x, enc, Wq, bq, Wk, bk, Wv, bv, Wp, aug_x, aug_e)
    res = run_spmd(in_maps, nc=nc)
    y = np.empty((B, N, C), np.float32)
    for b in range(B):
        y[b] = (res.results[2 * b]["ot"] +
                res.results[2 * b + 1]["ot"]).T + bp[None, :]
    return y
